# revision 1
# baseline (speedup 1.0000x reference)
"""Trainium2 Bass kernel for the quirky MultiHeadAttention module.

Reference computation (B=4, S=1024, H=768, NH=12, HS=64):
    Q = (x@Wq+bq)  split into heads     [B,12,S,64]
    K = (x@Wk+bk)  split into heads     [B,12,S,64]
    V = x@Wv+bv    NOT split            [B,S,768]
    A = softmax(QK^T/8 + mask)          [B,12,S,S]
    out = (A @ V) reshaped [B, S*12, H] @ Wo + bo    -> [4, 12288, 768]

Algebraic restructuring used here:
  * (A @ V) @ Wo = A @ (V @ Wo) = A @ (x @ (Wv@Wo) + 1x(bv@Wo)); with bo
    folded in, each output row is A[q,:] @ VW + c, c = bv@Wo + bo, and the
    +c term is realized exactly by adding a constant row to VW (softmax
    rows sum to one in exact correspondence with the sigma column below).
  * Masked keys produce exp(-1e9+s) == 0 in fp32 for every head and every
    query (the mask is [B,1,1,S]), identically in the reference, so masked
    keys are dropped entirely on the host and the key axis is compacted
    (~2x less attention work for a Bernoulli(1/2) mask).
  * The softmax denominator comes from a ones-column appended to VW, and
    exp needs no max-subtraction (scores are O(1) for this problem).

Sharding: 8 cores = 4 batches x 2 head-groups (6 heads each). Pure SPMD,
no collectives. Everything is computed in a transposed layout so no
on-device transposes are needed:
    QT/KT: [384 feat, tok] (head-pairs packed 64+64 in partitions; the
        64-row score matmuls are row-packed on the PE via tile_position)
    S^T = KT_h-slices.T @ QT_h  -> [k, q]  (k on partitions => the mask is
        a per-partition bias folded into the Exp activation for free)
    U = exp(S^T)  [k, q] fp16   -> exactly the layout the PV matmul needs
    O = U.T @ [VW | 1]  -> [q, 769] with col 768 = softmax denominator
Matmul operands are fp16 (same PE speed as bf16 on TRN2, ~4x less rounding
error); all accumulation is fp32 in PSUM. Inputs are packed host-side into
partition-major [128, N] blobs so the input DMAs run long contiguous lines,
and a burst of dummy matmuls during the initial DMA wait pre-warms the PE
clock (HAM) to 2.4 GHz.
"""

import math

import numpy as np

B, S, H, NH, HS = 4, 1024, 768, 12, 64
GW = 384          # head-group width = 6 heads * 64
NCORES = 8

_PROGRAM_CACHE = {}


def _pack6(a):
    """[768, N] -> partition-major [128, 6*N] (tile i at cols i*N:(i+1)*N)."""
    n = a.shape[1]
    return np.ascontiguousarray(
        a.reshape(6, 128, n).transpose(1, 0, 2).reshape(128, 6 * n))


def _build_program(kt_tiles, has_cvec):
    """kt_tiles: number of 128-wide compacted-key tiles (1..8).
    has_cvec: include the rank-1 (bv@Wo + bo) constant row in VW."""
    import concourse.mybir as mybir
    import concourse.tile as tile
    from concourse import bacc
    from concourse.bass import ds, ts

    f32 = mybir.dt.float32
    f16 = mybir.dt.float16
    AF = mybir.ActivationFunctionType

    KMAX = 128 * kt_tiles
    # key chunks (<=512 wide, balanced) for the KT projection
    if KMAX <= 512:
        kchunks = [(0, KMAX)]
    else:
        w1 = 128 * ((kt_tiles + 1) // 2)
        kchunks = [(0, w1), (w1, KMAX - w1)]

    nc = bacc.Bacc(None, target_bir_lowering=False, debug=False)

    xp_d = nc.dram_tensor("xp", (128, 6 * 1024), f16, kind="ExternalInput")
    wqp_d = nc.dram_tensor("wqp", (128, 6 * 384), f16, kind="ExternalInput")
    wkp_d = nc.dram_tensor("wkp", (128, 6 * 384), f16, kind="ExternalInput")
    wvp_d = nc.dram_tensor("wvp", (128, 6 * 768), f16, kind="ExternalInput")
    wvo6_d = nc.dram_tensor("wvo6", (1, 768), f16, kind="ExternalInput")
    # small fp32 per-partition vectors: cols = bq(3) bk(3) mk(kt_tiles)
    sv_d = nc.dram_tensor("sv", (128, 6 + kt_tiles), f32, kind="ExternalInput")
    out_d = nc.dram_tensor("out", (6, 1024, 768), f32, kind="ExternalOutput")

    with tile.TileContext(nc) as tc:
        with (
            tc.tile_pool(name="persist", bufs=1) as pp,
            tc.tile_pool(name="ut", bufs=4 * kt_tiles) as utp,
            tc.tile_pool(name="eps", bufs=8) as ep,
            tc.tile_pool(name="osb", bufs=4) as op_,
        ):
            # ---- stream inputs into SBUF (order = load priority) ----
            sv = pp.tile([128, 6 + kt_tiles], f32, name="sv", tag="sv")
            nc.sync.dma_start(sv[:], sv_d[:])
            bq_t = [sv[:, j:j + 1] for j in range(3)]
            bk_t = [sv[:, 3 + j:4 + j] for j in range(3)]
            mk_t = [sv[:, 6 + k:7 + k] for k in range(kt_tiles)]

            xbig = pp.tile([128, 6 * 1024], f16, name="xbig", tag="xbig")
            wqbig = pp.tile([128, 6 * 384], f16, name="wqbig", tag="wqbig")
            wkbig = pp.tile([128, 6 * 384], f16, name="wkbig", tag="wkbig")
            wvbig = pp.tile([128, 6 * 768], f16, name="wvbig", tag="wvbig")
            xkt6 = pp.tile([1, KMAX], f16, name="xkt6", tag="xkt6")
            wvo6 = pp.tile([1, 768], f16, name="wvo6", tag="wvo6")
            # Input loads: fine-grained pieces alternating over the two
            # HWDGE rings (sync, scalar) in consumption order, VW weights
            # on SWDGE (gpsimd). Small pieces land early so the first QT
            # matmuls can start while the rest of x streams in.
            rings = [nc.sync, nc.scalar]
            wh = 3 * 384
            for r in range(2):
                rings[r].dma_start(wqbig[:, r * wh:(r + 1) * wh],
                                   wqp_d[:, r * wh:(r + 1) * wh])
            for i in range(6):
                rings[i % 2].dma_start(xbig[:, i * 1024:(i + 1) * 1024],
                                       xp_d[:, i * 1024:(i + 1) * 1024])
            for r in range(2):
                rings[r].dma_start(wkbig[:, r * wh:(r + 1) * wh],
                                   wkp_d[:, r * wh:(r + 1) * wh])
            if has_cvec:
                nc.vector.memset(xkt6[:], 1.0)
                nc.scalar.dma_start(wvo6[:], wvo6_d[:])
            nc.gpsimd.dma_start(wvbig[:], wvp_d[:])

            xt = [xbig[:, i * 1024:(i + 1) * 1024] for i in range(6)]
            wq_t = [wqbig[:, i * 384:(i + 1) * 384] for i in range(6)]
            # tokens are host-permuted (kept keys first), so the K-side
            # tiles are just the leading columns of the same x buffer
            xkt = [xbig[:, i * 1024:i * 1024 + KMAX] for i in range(6)]
            wk_t = [wkbig[:, i * 384:(i + 1) * 384] for i in range(6)]
            wvo_t = [wvbig[:, i * 768:(i + 1) * 768] for i in range(6)]

            # persistent intermediates
            QT = [pp.tile([128, 1024], f16, name=f"QT{j}", tag=f"QT{j}")
                  for j in range(3)]
            KT = [pp.tile([128, KMAX], f16, name=f"KT{j}", tag=f"KT{j}")
                  for j in range(3)]
            VW = [pp.tile([128, 769], f16, name=f"VW{m}", tag=f"VW{m}")
                  for m in range(kt_tiles)]

            # ---- phase A: projections ----
            # PE warm-up: dummy matmuls on a tiny memset tile keep the
            # tensor engine active during the initial input DMA so the
            # HAM clock gate opens (2.4 GHz) before real work arrives.
            wsrc = pp.tile([1, 512], f16, name="wsrc", tag="wsrc")
            nc.vector.memset(wsrc[:], 0.0)
            with tc.tile_pool(name="psW", bufs=2, space="PSUM") as psW:
                for _ in range(12):
                    psw = psW.tile([1, 512], f32, name="warm", tag="warm")
                    nc.tensor.matmul(psw[:], wsrc[:, 0:1], wsrc[:])

            with tc.tile_pool(name="psA", bufs=6, space="PSUM") as psA:
                # QT is kt-major: all six (j,qc) PSUM groups accumulate in
                # parallel so each arriving x tile is consumed immediately
                # (no long PE stalls while x streams in).
                qgroups = [(j, qc) for j in range(3) for qc in range(2)]
                qps = [psA.tile([128, 512], f32, name=f"qtp{j}{qc}", tag="qk")
                       for j, qc in qgroups]
                for kt in range(6):
                    for gi, (j, qc) in enumerate(qgroups):
                        nc.tensor.matmul(
                            qps[gi][:], wq_t[kt][:, ts(j, 128)],
                            xt[kt][:, ds(qc * 512, 512)],
                            start=(kt == 0), stop=(kt == 5))
                for gi, (j, qc) in enumerate(qgroups):
                    nc.scalar.activation(
                        QT[j][:, ds(qc * 512, 512)], qps[gi][:], AF.Identity,
                        bias=bq_t[j])
                for j in range(3):
                    for o, w in kchunks:
                        kch = ds(o, w)
                        ps2 = psA.tile([128, 512], f32, name="ktp", tag="qk")
                        for kt in range(6):
                            nc.tensor.matmul(
                                ps2[:, 0:w], wk_t[kt][:, ts(j, 128)],
                                xkt[kt][:, kch],
                                start=(kt == 0), stop=(kt == 5))
                        nc.scalar.activation(
                            KT[j][:, kch], ps2[:, 0:w], AF.Identity,
                            bias=bk_t[j])
                for m in range(kt_tiles):   # compacted-key token tile
                    for ncn in range(2):    # output feature chunk of 384
                        fch = ds(ncn * 384, 384)
                        ps = psA.tile([128, 384], f32, name="vw", tag="vw",
                                      bufs=2)
                        for kt in range(6):
                            nc.tensor.matmul(
                                ps[:], xkt[kt][:, ts(m, 128)], wvo_t[kt][:, fch],
                                start=(kt == 0),
                                stop=(kt == 5 and not has_cvec))
                        if has_cvec:
                            nc.tensor.matmul(
                                ps[:], xkt6[:, ts(m, 128)], wvo6[:, fch],
                                start=False, stop=True)
                        nc.vector.tensor_copy(VW[m][:, fch], ps[:])
                    nc.vector.memset(VW[m][:, 768:769], 1.0)

            # ---- phase B: attention ----
            with (
                tc.tile_pool(name="psS", bufs=4, space="PSUM") as psSp,
                tc.tile_pool(name="psO", bufs=2, space="PSUM") as psOp,
            ):
                chunks = [(j, qc) for j in range(3) for qc in range(2)]

                def emit_scores(j, qc):
                    qch = ds(qc * 512, 512)
                    ut = [[None] * kt_tiles for _ in range(2)]
                    for kt in range(kt_tiles):
                        for hh in range(2):
                            p0 = hh * 64
                            ps = psSp.tile([128, 512], f32, name="psS",
                                           tag="psS")
                            # 64-row-packed scores^T: [k-tile, q-chunk]
                            nc.tensor.matmul(
                                ps[:],
                                KT[j][p0:p0 + 64, ts(kt, 128)],
                                QT[j][p0:p0 + 64, qch])
                            u = utp.tile([128, 512], f16, name="ut", tag="ut")
                            nc.scalar.activation(
                                u[:], ps[:], AF.Exp, bias=mk_t[kt])
                            ut[hh][kt] = u
                    return ut

                for ci, (j, qc) in enumerate(chunks):
                    ut = emit_scores(j, qc)
                    for gi, (hh, mq) in enumerate(
                            (hh, mq) for hh in range(2) for mq in range(4)):
                        head = j * 2 + hh
                        # 384+385 split: both PV chains stream ~160ns/MM so
                        # LDWEIGHTS (~97ns) stays fully hidden; sigma-chain
                        # first so the reciprocal overlaps the other chain.
                        pa = psOp.tile([128, 384], f32, name="psOa",
                                       tag="psOa")
                        pb = psOp.tile([128, 385], f32, name="psOb",
                                       tag="psOb")
                        for kt in range(kt_tiles):
                            nc.tensor.matmul(
                                pb[:], ut[hh][kt][:, ts(mq, 128)],
                                VW[kt][:, 384:769],
                                start=(kt == 0), stop=(kt == kt_tiles - 1))
                        for kt in range(kt_tiles):
                            nc.tensor.matmul(
                                pa[:], ut[hh][kt][:, ts(mq, 128)],
                                VW[kt][:, 0:384],
                                start=(kt == 0), stop=(kt == kt_tiles - 1))
                        rv = ep.tile([128, 1], f32, name="rinv", tag="rinv")
                        nc.vector.reciprocal(rv[:], pb[:, 384:385])
                        ob = op_.tile([128, 768], f32, name="ob", tag="ob")
                        orow = out_d[head, ds(qc * 512 + mq * 128, 128), :]
                        nc.vector.tensor_scalar_mul(
                            ob[:, 384:768], pb[:, 0:384], rv[:])
                        nc.scalar.dma_start(orow[:, 384:768], ob[:, 384:768])
                        nc.vector.tensor_scalar_mul(
                            ob[:, 0:384], pa[:], rv[:])
                        nc.sync.dma_start(orow[:, 0:384], ob[:, 0:384])
    nc.compile()
    return nc


def get_program(kt_tiles=8, has_cvec=True):
    key = (kt_tiles, has_cvec)
    if key not in _PROGRAM_CACHE:
        _PROGRAM_CACHE[key] = _build_program(*key)
    return _PROGRAM_CACHE[key]


def prep(x, mask, Wq, bq, Wk, bk, Wv, bv, Wo, bo):
    """Host-side sharding/compaction.
    Tokens are permuted per batch so unmasked keys come first; the device
    computes everything in permuted token order and gather_output undoes
    the permutation. Returns (kt_tiles, has_cvec, in_maps, perms)."""
    f16 = np.float16
    x = np.asarray(x, np.float32)
    mask = np.asarray(mask)
    Wq = np.asarray(Wq, np.float32)
    Wk = np.asarray(Wk, np.float32)
    Wv = np.asarray(Wv, np.float32)
    Wo = np.asarray(Wo, np.float32)
    bq = np.asarray(bq, np.float32)
    bk = np.asarray(bk, np.float32)
    bv = np.asarray(bv, np.float32)
    bo = np.asarray(bo, np.float32)

    mrow = [mask[b, 0, 0] != 0 for b in range(B)]
    perms = [np.argsort(~mrow[b], kind="stable") for b in range(B)]
    nkeep = [int(mrow[b].sum()) for b in range(B)]
    kt_tiles = min(8, max(1, math.ceil(max(nkeep) / 128)))
    KMAX = 128 * kt_tiles

    cvec = bv @ Wo + bo
    has_cvec = bool(np.any(cvec))

    # per-head-group packed weights (shared across the 4 batches)
    wq_p, wk_p, bq_p, bk_p = [], [], [], []
    for g in range(2):
        cs = slice(g * GW, (g + 1) * GW)
        wq_p.append(_pack6((Wq[:, cs] * 0.125).astype(f16)))
        wk_p.append(_pack6(Wk[:, cs].astype(f16)))
        bq_p.append((bq[cs] * 0.125).reshape(3, 128).T)   # [128,3]
        bk_p.append(bk[cs].reshape(3, 128).T)
    wvp = _pack6((Wv @ Wo).astype(f16))
    wvo6 = cvec.astype(f16).reshape(1, 768)

    xp_b, sv_b = [], []
    for b in range(B):
        xp_b.append(_pack6(x[b][perms[b]].T.astype(f16)))
        sv = np.empty((128, 6 + kt_tiles), np.float32)
        mk = np.full(KMAX, -1e9, np.float32)
        mk[:nkeep[b]] = 0.0
        sv[:, 6:] = mk.reshape(kt_tiles, 128).T
        sv_b.append(sv)

    in_maps = []
    for c in range(NCORES):
        b, g = c // 2, c % 2
        sv = sv_b[b].copy()
        sv[:, 0:3] = bq_p[g]
        sv[:, 3:6] = bk_p[g]
        in_maps.append({
            "xp": xp_b[b],
            "wqp": wq_p[g],
            "wkp": wk_p[g],
            "wvp": wvp,
            "wvo6": wvo6,
            "sv": sv,
        })
    return kt_tiles, has_cvec, in_maps, perms


def gather_output(results, perms):
    out = np.empty((B, S * NH, H), np.float32)
    ov = out.reshape(B, S, NH, H)
    for c in range(NCORES):
        b, g = c // 2, c % 2
        o = results[c]["out"]  # [6, 1024(permuted q), 768]
        ov[b, perms[b], g * 6:(g + 1) * 6, :] = o.transpose(1, 0, 2)
    return out


def kernel(**inputs):
    from concourse.bass_utils import run_bass_kernel_spmd

    kt_tiles, has_cvec, in_maps, perms = prep(**inputs)
    nc = get_program(kt_tiles, has_cvec)
    res = run_bass_kernel_spmd(nc, in_maps, core_ids=list(range(NCORES)))
    return gather_output(res.results, perms)


if __name__ == "__main__":
    rng = np.random.default_rng(0)
    demo = {
        "x": rng.standard_normal((B, S, H), dtype=np.float32),
        "mask": rng.integers(0, 2, (B, 1, 1, S)).astype(np.int32),
        "Wq": rng.standard_normal((H, H), dtype=np.float32) / np.sqrt(H),
        "bq": np.zeros(H, np.float32),
        "Wk": rng.standard_normal((H, H), dtype=np.float32) / np.sqrt(H),
        "bk": np.zeros(H, np.float32),
        "Wv": rng.standard_normal((H, H), dtype=np.float32) / np.sqrt(H),
        "bv": np.zeros(H, np.float32),
        "Wo": rng.standard_normal((H, H), dtype=np.float32) / np.sqrt(H),
        "bo": np.zeros(H, np.float32),
    }
    out = kernel(**demo)
    print("kernel ran, output shape", out.shape)



# revision 7
# speedup vs baseline: 1.0537x; 1.0537x over previous
"""Trainium2 Bass kernel for the quirky MultiHeadAttention module.

Reference computation (B=4, S=1024, H=768, NH=12, HS=64):
    Q = (x@Wq+bq)  split into heads     [B,12,S,64]
    K = (x@Wk+bk)  split into heads     [B,12,S,64]
    V = x@Wv+bv    NOT split            [B,S,768]
    A = softmax(QK^T/8 + mask)          [B,12,S,S]
    out = (A @ V) reshaped [B, S*12, H] @ Wo + bo    -> [4, 12288, 768]

Algebraic restructuring:
  * (A @ V) @ Wo = A @ (x @ (Wv@Wo)) (+ cvec = bv@Wo + bo, realized by a
    constant row added to VW -- softmax rows sum to one).
  * Masked keys produce exp(-1e9+s) == 0 exactly, so they are dropped on
    the host and the key axis is compacted per batch.
  * The softmax denominator comes from a ones-column appended to VW.
    The device emits UNNORMALIZED numerators + the sigma column in f16;
    the host performs the division (allows partial-sum sharding).

Sharding: 8 cores = 4 batches x 2 head-groups (6 heads each), pure SPMD.
Key tiles are 128 wide. The base program uses B_kt tiles per core; when
exactly one "tile level" of overflow exists (e.g. per-batch tile counts
[4,4,4,5]), the overflow (batch, head-pair, tile) units are farmed out
one-per-core as an "extra unit" (cores of the overflowing batch run a
fully-masked dummy), and the host sums the unnormalized partials. This
removes the padding waste of a uniform max-tile program.

Layouts (all transposed so no on-device transposes are needed):
    QT/KT: [feat, tok] with head pairs packed 64+64 in partitions; the
        64-row score matmuls run 2-head-concurrent via PE row groups.
    S^T = KT.T @ QT -> [k, q]  (k on partitions => mask is a per-partition
        bias folded into the Exp activation)
    U = exp(S^T) [k, q] f16 -> exactly the layout the PV matmul needs
    out = U.T @ [VW | 1] -> [q, 769] with col 768 = sigma, stored f16.
Matmul operands are f16 (same PE speed as bf16, ~4x less rounding error
than bf16); accumulation is f32 in PSUM. Inputs are packed host-side into
partition-major [128, N] blobs and streamed over all three DMA queues
(sync/scalar HWDGE + gpsimd SWDGE) in consumption order; a burst of tiny
matmuls pre-warms the PE clock gate (HAM) to 2.4 GHz before real work.
"""

import math

import numpy as np

B, S, H, NH, HS = 4, 1024, 768, 12, 64
GW = 384          # head-group width = 6 heads * 64
NCORES = 8

_PROGRAM_CACHE = {}


def _pack6(a):
    """[768, N] -> partition-major [128, 6*N] (tile i at cols i*N:(i+1)*N)."""
    n = a.shape[1]
    return np.ascontiguousarray(
        a.reshape(6, 128, n).transpose(1, 0, 2).reshape(128, 6 * n))


def _build_program(bkt, has_extra, has_cvec):
    """bkt: number of base 128-wide key tiles per core (1..8).
    has_extra: include one (pair, tile) overflow unit per core.
    has_cvec: include the rank-1 (bv@Wo + bo) constant row in VW."""
    import concourse.mybir as mybir
    import concourse.tile as tile
    from concourse import bacc
    from concourse.bass import ds, ts

    f32 = mybir.dt.float32
    f16 = mybir.dt.float16
    AF = mybir.ActivationFunctionType

    KMAX = 128 * bkt
    if KMAX <= 512:
        kchunks = [(0, KMAX)]
    else:
        w1 = 128 * ((bkt + 1) // 2)
        kchunks = [(0, w1), (w1, KMAX - w1)]
    # sv columns: bq(3) bk(3) mk(bkt) [mke bqe bke]
    nsv = 6 + bkt + (3 if has_extra else 0)

    nc = bacc.Bacc(None, target_bir_lowering=False, debug=False)

    xp_d = nc.dram_tensor("xp", (128, 6 * 1024), f16, kind="ExternalInput")
    wqp_d = nc.dram_tensor("wqp", (128, 6 * 384), f16, kind="ExternalInput")
    wkp_d = nc.dram_tensor("wkp", (128, 6 * 384), f16, kind="ExternalInput")
    wvp_d = nc.dram_tensor("wvp", (128, 6 * 768), f16, kind="ExternalInput")
    sv_d = nc.dram_tensor("sv", (128, nsv), f32, kind="ExternalInput")
    if has_cvec:
        wvo6_d = nc.dram_tensor("wvo6", (1, 768), f16, kind="ExternalInput")
    if has_extra:
        xe_d = nc.dram_tensor("xe", (128, 6 * 1024), f16, kind="ExternalInput")
        xekt_d = nc.dram_tensor("xekt", (128, 6 * 128), f16,
                                kind="ExternalInput")
        wqe_d = nc.dram_tensor("wqe", (128, 6 * 128), f16,
                               kind="ExternalInput")
        wke_d = nc.dram_tensor("wke", (128, 6 * 128), f16,
                               kind="ExternalInput")
    out_d = nc.dram_tensor("out", (3, 1024, 2, 769), f16,
                           kind="ExternalOutput")
    if has_extra:
        oute_d = nc.dram_tensor("oute", (1024, 2, 769), f16,
                                kind="ExternalOutput")

    with tile.TileContext(nc) as tc:
        with (
            tc.tile_pool(name="persist", bufs=1) as pp,
            tc.tile_pool(name="ut", bufs=4 * max(bkt, 2)) as utp,
            tc.tile_pool(name="osb", bufs=6) as op_,
        ):
            # ---- stream inputs (order within each queue = priority) ----
            sv = pp.tile([128, nsv], f32, name="sv", tag="sv")
            nc.sync.dma_start(sv[:], sv_d[:])
            bq_t = [sv[:, j:j + 1] for j in range(3)]
            bk_t = [sv[:, 3 + j:4 + j] for j in range(3)]
            mk_t = [sv[:, 6 + k:7 + k] for k in range(bkt)]
            if has_extra:
                mke_t = sv[:, 6 + bkt:7 + bkt]
                bqe_t = sv[:, 7 + bkt:8 + bkt]
                bke_t = sv[:, 8 + bkt:9 + bkt]

            xbig = pp.tile([128, 6 * 1024], f16, name="xbig", tag="xbig")
            wqbig = pp.tile([128, 6 * 384], f16, name="wqbig", tag="wqbig")
            wkbig = pp.tile([128, 6 * 384], f16, name="wkbig", tag="wkbig")
            wvbig = pp.tile([128, 6 * 768], f16, name="wvbig", tag="wvbig")
            if has_extra:
                xebig = pp.tile([128, 6 * 1024], f16, name="xebig",
                                tag="xebig")
                xekt = pp.tile([128, 6 * 128], f16, name="xekt", tag="xekt")
                wqe = pp.tile([128, 6 * 128], f16, name="wqe", tag="wqe")
                wke = pp.tile([128, 6 * 128], f16, name="wke", tag="wke")
            if has_cvec:
                ones1 = pp.tile([1, 128], f16, name="ones1", tag="ones1")
                wvo6 = pp.tile([1, 768], f16, name="wvo6", tag="wvo6")

            # sync: wq even + x 0,3 + wk even (+wqe); scalar: wq odd +
            # x 1,4 + wk odd (+wke); gpsimd(SWDGE): x 2,5 + xekt + wvp + xe.
            def wq_piece(i):
                return (wqbig[:, i * 384:(i + 1) * 384],
                        wqp_d[:, i * 384:(i + 1) * 384])

            def wk_piece(i):
                return (wkbig[:, i * 384:(i + 1) * 384],
                        wkp_d[:, i * 384:(i + 1) * 384])

            def x_piece(i):
                return (xbig[:, i * 1024:(i + 1) * 1024],
                        xp_d[:, i * 1024:(i + 1) * 1024])

            nc.sync.dma_start(*wq_piece(0))
            nc.scalar.dma_start(*wq_piece(1))
            nc.gpsimd.dma_start(*x_piece(2))
            nc.sync.dma_start(*x_piece(0))
            nc.scalar.dma_start(*x_piece(1))
            nc.gpsimd.dma_start(*x_piece(5))
            nc.sync.dma_start(*wq_piece(2))
            nc.scalar.dma_start(*wq_piece(3))
            nc.sync.dma_start(*x_piece(3))
            nc.scalar.dma_start(*x_piece(4))
            nc.sync.dma_start(*wq_piece(4))
            nc.scalar.dma_start(*wq_piece(5))
            for i in range(6):
                (nc.sync if i % 2 == 0 else nc.scalar).dma_start(*wk_piece(i))
            if has_extra:
                nc.gpsimd.dma_start(xekt[:], xekt_d[:])
                nc.sync.dma_start(wqe[:], wqe_d[:])
                nc.scalar.dma_start(wke[:], wke_d[:])
            if has_cvec:
                nc.vector.memset(ones1[:], 1.0)
                nc.scalar.dma_start(wvo6[:], wvo6_d[:])
            nc.gpsimd.dma_start(wvbig[:], wvp_d[:])
            if has_extra:
                nc.gpsimd.dma_start(xebig[:], xe_d[:])

            xt = [xbig[:, i * 1024:(i + 1) * 1024] for i in range(6)]
            wq_t = [wqbig[:, i * 384:(i + 1) * 384] for i in range(6)]
            xkt = [xbig[:, i * 1024:i * 1024 + KMAX] for i in range(6)]
            wk_t = [wkbig[:, i * 384:(i + 1) * 384] for i in range(6)]
            wvo_t = [wvbig[:, i * 768:(i + 1) * 768] for i in range(6)]
            if has_extra:
                xet = [xebig[:, i * 1024:(i + 1) * 1024] for i in range(6)]

            # persistent intermediates
            QT = [pp.tile([128, 1024], f16, name=f"QT{j}", tag=f"QT{j}")
                  for j in range(3)]
            KT = [pp.tile([128, KMAX], f16, name=f"KT{j}", tag=f"KT{j}")
                  for j in range(3)]
            VW = [pp.tile([128, 769], f16, name=f"VW{m}", tag=f"VW{m}")
                  for m in range(bkt)]
            if has_extra:
                QTe = pp.tile([128, 1024], f16, name="QTe", tag="QTe")
                KTe = pp.tile([128, 128], f16, name="KTe", tag="KTe")
                VWe = pp.tile([128, 769], f16, name="VWe", tag="VWe")

            # ---- phase A: projections ----
            # PE warm-up: small junk matmuls keep the tensor engine busy
            # through the HAM SHORT window (~3.4us) so real work runs at
            # 2.4 GHz from the start.
            wsrc = pp.tile([1, 128], f16, name="wsrc", tag="wsrc")
            nc.vector.memset(wsrc[:], 0.0)
            with tc.tile_pool(name="psW", bufs=2, space="PSUM") as psW:
                for _ in range(36):
                    psw = psW.tile([1, 128], f32, name="warm", tag="warm")
                    nc.tensor.matmul(psw[:], wsrc[:, 0:1], wsrc[:])

            with tc.tile_pool(name="psA", bufs=6, space="PSUM") as psA:
                # QT is kt-major: all six (j,qc) PSUM groups accumulate in
                # parallel so each arriving x tile is consumed immediately.
                qgroups = [(j, qc) for j in range(3) for qc in range(2)]
                qps = [psA.tile([128, 512], f32, name=f"qtp{j}{qc}", tag="qk")
                       for j, qc in qgroups]
                for kt in range(6):
                    for gi, (j, qc) in enumerate(qgroups):
                        nc.tensor.matmul(
                            qps[gi][:], wq_t[kt][:, ts(j, 128)],
                            xt[kt][:, ds(qc * 512, 512)],
                            start=(kt == 0), stop=(kt == 5))
                for gi, (j, qc) in enumerate(qgroups):
                    nc.scalar.activation(
                        QT[j][:, ds(qc * 512, 512)], qps[gi][:], AF.Identity,
                        bias=bq_t[j])
                for j in range(3):
                    for o, w in kchunks:
                        kch = ds(o, w)
                        ps2 = psA.tile([128, 512], f32, name="ktp", tag="qk")
                        for kt in range(6):
                            nc.tensor.matmul(
                                ps2[:, 0:w], wk_t[kt][:, ts(j, 128)],
                                xkt[kt][:, kch],
                                start=(kt == 0), stop=(kt == 5))
                        nc.scalar.activation(
                            KT[j][:, kch], ps2[:, 0:w], AF.Identity,
                            bias=bk_t[j])
                if has_extra:
                    # extra pair's K^T over its single key tile
                    pse = psA.tile([128, 128], f32, name="kte", tag="qk")
                    for kt in range(6):
                        nc.tensor.matmul(
                            pse[:], wke[:, ts(kt, 128)],
                            xekt[:, ts(kt, 128)],
                            start=(kt == 0), stop=(kt == 5))
                    nc.scalar.activation(KTe[:], pse[:], AF.Identity,
                                         bias=bke_t)
                    # extra pair's Q^T over all 1024 extra-batch tokens
                    for qc in range(2):
                        psq = psA.tile([128, 512], f32, name="qte", tag="qk")
                        for kt in range(6):
                            nc.tensor.matmul(
                                psq[:], wqe[:, ts(kt, 128)],
                                xet[kt][:, ds(qc * 512, 512)],
                                start=(kt == 0), stop=(kt == 5))
                        nc.scalar.activation(
                            QTe[:, ds(qc * 512, 512)], psq[:], AF.Identity,
                            bias=bqe_t)

                def emit_vw(dst, src_tiles, msel):
                    # dst[k,f] accumulates src.T @ (Wv@Wo) for one key tile
                    for ncn in range(2):
                        fch = ds(ncn * 384, 384)
                        ps = psA.tile([128, 384], f32, name="vw", tag="vw",
                                      bufs=2)
                        for kt in range(6):
                            nc.tensor.matmul(
                                ps[:], src_tiles[kt][:, msel],
                                wvo_t[kt][:, fch],
                                start=(kt == 0),
                                stop=(kt == 5 and not has_cvec))
                        if has_cvec:
                            nc.tensor.matmul(
                                ps[:], ones1[:], wvo6[:, fch],
                                start=False, stop=True)
                        nc.vector.tensor_copy(dst[:, fch], ps[:])
                    nc.vector.memset(dst[:, 768:769], 1.0)

                for m in range(bkt):
                    emit_vw(VW[m], xkt, ts(m, 128))
                if has_extra:
                    emit_vw(VWe, [xekt[:, ts(i, 128)] for i in range(6)],
                            ds(0, 128))

            # ---- phase B: attention ----
            rings = [nc.sync, nc.scalar, nc.gpsimd]
            ring_i = [0]

            def out_dma(dst, src):
                rings[ring_i[0] % 3].dma_start(dst, src)
                ring_i[0] += 1

            with (
                tc.tile_pool(name="psS", bufs=4, space="PSUM") as psSp,
                tc.tile_pool(name="psO", bufs=2, space="PSUM") as psOp,
            ):
                def emit_chunk(qch, kt_sb, qt_sb, masks, odst, small):
                    """One (pair-like, q-chunk) unit: scores+exp+PV+out.
                    kt_sb[i] = (KT tile, col slice, VW tile). qt_sb: QT
                    [128,1024] tile. masks: per-tile bias APs. odst: HBM
                    [128 q, 2, 769] f16 destination."""
                    nkt = len(kt_sb)
                    ut = [[None] * nkt for _ in range(2)]
                    for i in range(nkt):
                        ktile, csel, _vw = kt_sb[i]
                        for hh in range(2):
                            p0 = hh * 64
                            ps = psSp.tile([128, 512], f32, name="psS",
                                           tag="psS")
                            nc.tensor.matmul(
                                ps[:], ktile[p0:p0 + 64, csel],
                                qt_sb[p0:p0 + 64, qch])
                            u = utp.tile([128, 512], f16, name="ut", tag="ut")
                            nc.scalar.activation(
                                u[:], ps[:], AF.Exp, bias=masks[i])
                            ut[hh][i] = u
                    ob = [op_.tile([128, 1538], f16, name="ob", tag="ob")
                          for _ in range(4)]
                    for hh in range(2):
                        for mq in range(4):
                            pa = psOp.tile([128, 384], f32, name="psOa",
                                           tag="psOa")
                            pb = psOp.tile([128, 385], f32, name="psOb",
                                           tag="psOb")
                            for i in range(nkt):
                                nc.tensor.matmul(
                                    pb[:], ut[hh][i][:, ts(mq, 128)],
                                    kt_sb[i][2][:, 384:769],
                                    start=(i == 0), stop=(i == nkt - 1))
                            for i in range(nkt):
                                nc.tensor.matmul(
                                    pa[:], ut[hh][i][:, ts(mq, 128)],
                                    kt_sb[i][2][:, 0:384],
                                    start=(i == 0), stop=(i == nkt - 1))
                            oh = ob[mq][:, ds(hh * 769, 769)]
                            # PSUM reads: only DVE/ACT may touch PSUM;
                            # split casts between them to balance load
                            nc.vector.tensor_copy(oh[:, 0:384], pa[:])
                            if small or hh == 1:
                                nc.scalar.copy(oh[:, 384:769], pb[:])
                            else:
                                nc.vector.tensor_copy(oh[:, 384:769], pb[:])
                    for mq in range(4):
                        out_dma(odst[mq], ob[mq][:])

                # extra-unit chunks interleave after the first two base
                # chunks so their epilogue (casts/DMA) hides under base PV
                for ci, (j, qc) in enumerate(
                        (j, qc) for j in range(3) for qc in range(2)):
                    odst = [out_d[j, ds(qc * 512 + mq * 128, 128), :, :]
                            for mq in range(4)]
                    emit_chunk(
                        ds(qc * 512, 512),
                        [(KT[j], ts(kt, 128), VW[kt]) for kt in range(bkt)],
                        qt_sb=QT[j], masks=mk_t, odst=odst, small=False)
                    if has_extra and ci < 2:
                        eodst = [oute_d[ds(ci * 512 + mq * 128, 128), :, :]
                                 for mq in range(4)]
                        emit_chunk(
                            ds(ci * 512, 512), [(KTe, ds(0, 128), VWe)],
                            qt_sb=QTe, masks=[mke_t], odst=eodst, small=True)
    nc.compile()
    return nc


def get_program(bkt, has_extra, has_cvec):
    key = (bkt, has_extra, has_cvec)
    if key not in _PROGRAM_CACHE:
        _PROGRAM_CACHE[key] = _build_program(*key)
    return _PROGRAM_CACHE[key]


def _pair_cols(w, pair, scale=1.0):
    """Wq/Wk columns for one head pair -> packed [128, 6*128] f16."""
    cs = w[:, pair * 128:(pair + 1) * 128] * scale
    return _pack6(cs.astype(np.float16))


def prep(x, mask, Wq, bq, Wk, bk, Wv, bv, Wo, bo):
    """Host-side sharding/compaction. Returns (bkt, has_extra, has_cvec,
    in_maps, perms, extras) where extras[c] = (batch, pair) or None."""
    f16 = np.float16
    x = np.asarray(x, np.float32)
    mask = np.asarray(mask)
    Wq = np.asarray(Wq, np.float32)
    Wk = np.asarray(Wk, np.float32)
    Wv = np.asarray(Wv, np.float32)
    Wo = np.asarray(Wo, np.float32)
    bq = np.asarray(bq, np.float32)
    bk = np.asarray(bk, np.float32)
    bv = np.asarray(bv, np.float32)
    bo = np.asarray(bo, np.float32)

    mrow = [mask[b, 0, 0] != 0 for b in range(B)]
    perms = [np.argsort(~mrow[b], kind="stable") for b in range(B)]
    nkeep = [int(mrow[b].sum()) for b in range(B)]
    tb = [min(8, max(1, math.ceil(n / 128))) for n in nkeep]
    tmax = max(tb)
    # overflow (batch, pair) units if base = tmax-1; one extra slot per
    # core when the total fits in 8, else fall back to the uniform max.
    ov = [(b, p) for b in range(B) if tb[b] == tmax for p in range(6)]
    if tmax > 1 and 0 < len(ov) <= NCORES and min(tb) < tmax:
        bkt, has_extra = tmax - 1, True
    else:
        bkt, has_extra = tmax, False
        ov = []
    KMAX = 128 * bkt

    cvec = bv @ Wo + bo
    has_cvec = bool(np.any(cvec))

    wq_p, wk_p, bq_p, bk_p = [], [], [], []
    for g in range(2):
        cs = slice(g * GW, (g + 1) * GW)
        wq_p.append(_pack6((Wq[:, cs] * 0.125).astype(f16)))
        wk_p.append(_pack6(Wk[:, cs].astype(f16)))
        bq_p.append((bq[cs] * 0.125).reshape(3, 128).T)   # [128,3]
        bk_p.append(bk[cs].reshape(3, 128).T)
    wvp = _pack6((Wv @ Wo).astype(f16))
    wvo6 = cvec.astype(f16).reshape(1, 768)

    xp_b, mk_b = [], []
    for b in range(B):
        xp_b.append(_pack6(x[b][perms[b]].T.astype(f16)))
        mk = np.full(KMAX, -1e9, np.float32)
        mk[:min(nkeep[b], KMAX)] = 0.0
        mk_b.append(mk.reshape(bkt, 128).T)

    # extra-slot assignment: overflow units go to non-owner cores first
    extras = [None] * NCORES
    if has_extra:
        order = sorted(range(NCORES), key=lambda c: c // 2 in
                       {b for b, _ in ov})
        for slot, unit in zip(order, ov):
            extras[slot] = unit

    def xe_tile(b):
        """Packed [128, 6*128] of the extra key-tile tokens of batch b."""
        lo = bkt * 128
        xs = np.zeros((128, 768), np.float32)
        hi = min(1024, lo + 128)
        xs[:hi - lo] = x[b][perms[b][lo:hi]]
        return _pack6(np.ascontiguousarray(xs.T).astype(f16))

    nsv = 6 + bkt + (3 if has_extra else 0)
    in_maps = []
    for c in range(NCORES):
        b, g = c // 2, c % 2
        sv = np.zeros((128, nsv), np.float32)
        sv[:, 0:3] = bq_p[g]
        sv[:, 3:6] = bk_p[g]
        sv[:, 6:6 + bkt] = mk_b[b]
        im = {"xp": xp_b[b], "wqp": wq_p[g], "wkp": wk_p[g], "wvp": wvp}
        if has_cvec:
            im["wvo6"] = wvo6
        if has_extra:
            if extras[c] is not None:
                eb, ep = extras[c]
                mke = np.full(128, -1e9, np.float32)
                ner = nkeep[eb] - bkt * 128
                mke[:max(0, min(128, ner))] = 0.0
                sv[:, 6 + bkt] = mke
                sv[:, 7 + bkt] = bq[ep * 128:(ep + 1) * 128] * 0.125
                sv[:, 8 + bkt] = bk[ep * 128:(ep + 1) * 128]
                im["xe"] = xp_b[eb]
                im["xekt"] = xe_tile(eb)
                im["wqe"] = _pair_cols(Wq, ep, 0.125)
                im["wke"] = _pair_cols(Wk, ep)
            else:
                sv[:, 6 + bkt] = -1e9   # dummy: fully masked
                im["xe"] = xp_b[b]
                im["xekt"] = xe_tile(b)
                im["wqe"] = _pair_cols(Wq, 0, 0.125)
                im["wke"] = _pair_cols(Wk, 0)
        im["sv"] = sv
        in_maps.append(im)
    return bkt, has_extra, has_cvec, in_maps, perms, extras


def gather_output(results, perms, extras):
    num = np.zeros((B, NH, S, 768), np.float32)
    sig = np.zeros((B, NH, S, 1), np.float32)
    for c in range(NCORES):
        b, g = c // 2, c % 2
        o = np.asarray(results[c]["out"], np.float32)  # [3,1024,2,769]
        for j in range(3):
            for hh in range(2):
                h = g * 6 + j * 2 + hh
                num[b, h] += o[j, :, hh, :768]
                sig[b, h, :, 0] += o[j, :, hh, 768]
        if extras[c] is not None:
            eb, ep = extras[c]
            oe = np.asarray(results[c]["oute"], np.float32)  # [1024,2,769]
            for hh in range(2):
                h = ep * 2 + hh
                num[eb, h] += oe[:, hh, :768]
                sig[eb, h, :, 0] += oe[:, hh, 768]
    res = num / sig                                    # [B,NH,S,H]
    out = np.empty((B, S * NH, H), np.float32)
    ov = out.reshape(B, S, NH, H)
    for b in range(B):
        ov[b, perms[b]] = res[b].transpose(1, 0, 2)
    return out


def kernel(**inputs):
    from concourse.bass_utils import run_bass_kernel_spmd

    bkt, has_extra, has_cvec, in_maps, perms, extras = prep(**inputs)
    nc = get_program(bkt, has_extra, has_cvec)
    res = run_bass_kernel_spmd(nc, in_maps, core_ids=list(range(NCORES)))
    return gather_output(res.results, perms, extras)


if __name__ == "__main__":
    rng = np.random.default_rng(0)
    demo = {
        "x": rng.standard_normal((B, S, H), dtype=np.float32),
        "mask": rng.integers(0, 2, (B, 1, 1, S)).astype(np.int32),
        "Wq": rng.standard_normal((H, H), dtype=np.float32) / np.sqrt(H),
        "bq": np.zeros(H, np.float32),
        "Wk": rng.standard_normal((H, H), dtype=np.float32) / np.sqrt(H),
        "bk": np.zeros(H, np.float32),
        "Wv": rng.standard_normal((H, H), dtype=np.float32) / np.sqrt(H),
        "bv": np.zeros(H, np.float32),
        "Wo": rng.standard_normal((H, H), dtype=np.float32) / np.sqrt(H),
        "bo": np.zeros(H, np.float32),
    }
    out = kernel(**demo)
    print("kernel ran, output shape", out.shape)
